# revision 1
# baseline (speedup 1.0000x reference)
"""Trainium2 Bass kernel: fused multi-head causal self-attention block.

Computes, for x:(B,S,H), W_qkv:(3H,H), b_qkv:(3H,), W_out:(H,H), b_out:(H,):
    qkv = x @ W_qkv.T + b_qkv ; split into q,k,v heads (NH heads, D=H/NH)
    out = softmax(causal(q k^T / sqrt(D))) v   ; merge heads
    return out @ W_out.T + b_out

Sharding over 8 NeuronCores: DP(2 batches) x TP(4 head-groups).
Core c handles batch b=c//4, head group g=c%4 (heads 4g..4g+3).
After per-head attention, the per-head outputs (stored transposed, [D,S])
are AllGather'd within each batch group of 4 cores; each core then computes
a disjoint 512-column slice of the output projection, so the host does a
pure concatenation (no host-side arithmetic beyond layout).

All device matmuls run as float32r (full-rate PE path) by default; storage
and accumulation are fp32.
"""

import math

import numpy as np

import concourse.bass as bass
import concourse.mybir as mybir
import concourse.tile as tile
from concourse import bacc
from concourse.bass_utils import run_bass_kernel_spmd

FP = mybir.dt.float32
FR = mybir.dt.float32r
F16 = mybir.dt.float16

# Full-size problem constants.
B, S, H, NH = 2, 2048, 2048, 16
D = 128
NCORES = 8
GROUPS = 4                  # head-groups per batch (TP degree)
REPLICA_GROUPS = [[0, 1, 2, 3], [4, 5, 6, 7]]

USE_F32R = True             # float32r matmuls (1 cyc/row) vs float32 (4 cyc/row)
MM_DT = FR if USE_F32R else FP
TRACE = False               # set by test harness to capture NTFF profile
LAST_EXEC_NS = None
LAST_RESULTS = None


def build_nc(s=S, h=H, nh=NH, reps=1, ag=True):
    """Build the SPMD Bass program (identical on all 8 cores).

    reps>1 repeats the whole computation in one NEFF; used only by the
    timing harness ((T(K)-T(1))/(K-1) cancels the dispatch overhead).
    """
    nl = nh // GROUPS           # local heads per core
    dg = nl * D                 # per-core slice of the head dim
    hc = h // 128               # contraction chunks for the projections
    sq = s // 512               # 512-wide q strips
    st_n = s // 128             # 128-row s tiles
    scale = 1.0 / math.sqrt(D)

    nc = bacc.Bacc(
        "TRN2",
        target_bir_lowering=False,
        debug=False,
        enable_asserts=False,
        num_devices=NCORES,
    )

    # ---- I/O -----------------------------------------------------------
    xT_d = nc.dram_tensor("xT", [h, s], MM_DT, kind="ExternalInput")
    wq_d = nc.dram_tensor("wq", [h, dg], MM_DT, kind="ExternalInput")
    wk_d = nc.dram_tensor("wk", [h, dg], MM_DT, kind="ExternalInput")
    wv_d = nc.dram_tensor("wv", [h, dg], MM_DT, kind="ExternalInput")
    wo_d = nc.dram_tensor("wo", [h, dg], F16, kind="ExternalInput")
    bq_d = nc.dram_tensor("bq", [128, nl], FP, kind="ExternalInput")
    bk_d = nc.dram_tensor("bk", [128, nl], FP, kind="ExternalInput")
    bv_d = nc.dram_tensor("bv", [128, dg], FP, kind="ExternalInput")
    bo_d = nc.dram_tensor("bo", [128, dg], FP, kind="ExternalInput")
    mask_d = nc.dram_tensor("mask", [128, 896], FP, kind="ExternalInput")
    ones_d = nc.dram_tensor("ones", [128, 128], MM_DT, kind="ExternalInput")
    out_d = nc.dram_tensor("out", [s, dg], FP, kind="ExternalOutput")

    with tile.TileContext(nc) as tc:
        with tc.tile_pool(name="const", bufs=1) as constp:
            mask_sb = constp.tile([128, 896], FP)
            nc.sync.dma_start(mask_sb[:], mask_d[:])
            ones_sb = constp.tile([128, 128], MM_DT)
            onesf_sb = constp.tile([1, 128], FP)
            nc.vector.memset(onesf_sb[:], 1.0)
            nc.sync.dma_start(ones_sb[:], ones_d[:])
            bq_sb = constp.tile([128, nl], FP)
            nc.sync.dma_start(bq_sb[:], bq_d[:])
            bk_sb = constp.tile([128, nl], FP)
            nc.sync.dma_start(bk_sb[:], bk_d[:])
            bv_sb = constp.tile([128, dg], FP)
            nc.sync.dma_start(bv_sb[:], bv_d[:])
            bo_sb = constp.tile([128, dg], FP)
            nc.sync.dma_start(bo_sb[:], bo_d[:])
            ones_col = ones_sb[:, 0:1]        # [128,1] lhsT for denominator sum
            ones_row = onesf_sb[0:1, :]       # [1,128] fp32 lhsT for partition-broadcast

            for _rep in range(reps):
                _emit_body(nc, tc, s, h, nh,
                           xT_d, wq_d, wk_d, wv_d, wo_d, out_d,
                           bq_sb, bk_sb, bv_sb, bo_sb,
                           mask_sb, ones_col, ones_row, scale, ag)

    nc.compile()
    return nc


def _emit_body(nc, tc, s, h, nh,
               xT_d, wq_d, wk_d, wv_d, wo_d, out_d,
               bq_sb, bk_sb, bv_sb, bo_sb,
               mask_sb, ones_col, ones_row, scale, ag=True):
    nl = nh // GROUPS
    dg = nl * D
    hc = h // 128               # 128-row contraction chunks
    hb_n = hc // 4              # batched (4-chunk) DMA groups
    sq = s // 512
    st_n = s // 128
    with tc.tile_pool(name="qkv", bufs=1) as qkvp:
        qT = [qkvp.tile([128, s], MM_DT, tag=f"qT{t}", name=f"qT{t}") for t in range(nl)]
        kT = [qkvp.tile([128, s], MM_DT, tag=f"kT{t}", name=f"kT{t}") for t in range(nl)]
        vv = [qkvp.tile([128, dg], MM_DT, tag=f"v{t}", name=f"v{t}") for t in range(st_n)]

        # ---- Phase A1: Q^T and K^T projections ------------------
        with tc.tile_pool(name="wqk", bufs=1) as wqkp, \
             tc.tile_pool(name="xA", bufs=3) as xap, \
             tc.tile_pool(name="psA", bufs=1, space="PSUM") as psA:
            wq_sb = [wqkp.tile([128, 4, dg], MM_DT, tag=f"wq{hb}", name=f"wq{hb}") for hb in range(hb_n)]
            wk_sb = [wqkp.tile([128, 4, dg], MM_DT, tag=f"wk{hb}", name=f"wk{hb}") for hb in range(hb_n)]
            for strip in range(sq):
                cs = slice(512 * strip, 512 * strip + 512)
                pss = [psA.tile([128, 512], FP, tag=f"psqk{gi}", name=f"psqk{gi}")
                       for gi in range(2 * nl)]
                for hb in range(hb_n):
                    xch = xap.tile([128, 4, 512], MM_DT, tag="xch", name="xch")
                    nc.sync.dma_start(
                        xch[:],
                        xT_d[512 * hb:512 * hb + 512, cs].rearrange("(c p) t -> p c t", p=128))
                    if strip == 0:  # interleave weight loads with first x stream
                        rows = slice(512 * hb, 512 * hb + 512)
                        nc.sync.dma_start(wq_sb[hb][:], wq_d[rows, :].rearrange("(c p) d -> p c d", p=128))
                        nc.sync.dma_start(wk_sb[hb][:], wk_d[rows, :].rearrange("(c p) d -> p c d", p=128))
                    for c in range(4):
                        hh = 4 * hb + c
                        for gi in range(2 * nl):
                            w_sb = wq_sb if gi < nl else wk_sb
                            t = gi % nl
                            nc.tensor.matmul(
                                pss[gi][:],
                                w_sb[hb][:, c, 128 * t:128 * t + 128],
                                xch[:, c, :],
                                start=(hh == 0), stop=(hh == hc - 1),
                            )
                for gi in range(2 * nl):
                    dstT = qT if gi < nl else kT
                    bias = bq_sb if gi < nl else bk_sb
                    t = gi % nl
                    nc.scalar.activation(
                        dstT[t][:, cs], pss[gi][:],
                        mybir.ActivationFunctionType.Identity,
                        bias=bias[:, t:t + 1],
                    )

        # ---- Phase A2: V projection (natural [s, d] layout) -----
        with tc.tile_pool(name="wvp", bufs=1) as wvp, \
             tc.tile_pool(name="xV", bufs=3) as xvp, \
             tc.tile_pool(name="psV", bufs=2, space="PSUM") as psV:
            wv_sb = [wvp.tile([128, 4, dg], MM_DT, tag=f"wv{hb}", name=f"wv{hb}") for hb in range(hb_n)]
            for strip in range(sq):
                cs = slice(512 * strip, 512 * strip + 512)
                psv = [psV.tile([128, dg], FP, tag=f"psv{sti}", name=f"psv{sti}")
                       for sti in range(4)]
                for hb in range(hb_n):
                    xch2 = xvp.tile([128, 4, 512], MM_DT, tag="xch2", name="xch2")
                    nc.sync.dma_start(
                        xch2[:],
                        xT_d[512 * hb:512 * hb + 512, cs].rearrange("(c p) t -> p c t", p=128))
                    if strip == 0:
                        rows = slice(512 * hb, 512 * hb + 512)
                        nc.sync.dma_start(wv_sb[hb][:], wv_d[rows, :].rearrange("(c p) d -> p c d", p=128))
                    for c in range(4):
                        hh = 4 * hb + c
                        for sti in range(4):
                            nc.tensor.matmul(
                                psv[sti][:],
                                xch2[:, c, 128 * sti:128 * sti + 128],
                                wv_sb[hb][:, c, :],
                                start=(hh == 0), stop=(hh == hc - 1),
                            )
                for sti in range(4):
                    nc.vector.tensor_add(vv[4 * strip + sti][:], psv[sti][:], bv_sb[:])

        # ---- Phase B + C: attention, AllGather, overlapped out-proj ----
        with tc.tile_pool(name="wop", bufs=1) as wop, \
             tc.tile_pool(name="etp", bufs=5) as etp, \
             tc.tile_pool(name="atp", bufs=3) as atp, \
             tc.tile_pool(name="rbp", bufs=2) as rbp, \
             tc.tile_pool(name="oaccp", bufs=1) as oaccp, \
             tc.tile_pool(name="atsp", bufs=4) as atsp, \
             tc.tile_pool(name="outp", bufs=2) as outp, \
             tc.tile_pool(name="dramp", bufs=1, space="DRAM") as dramp, \
             tc.tile_pool(name="psS", bufs=2, space="PSUM") as psS, \
             tc.tile_pool(name="psAV", bufs=2, space="PSUM") as psAV, \
             tc.tile_pool(name="psDN", bufs=1, space="PSUM") as psDN, \
             tc.tile_pool(name="psO", bufs=2, space="PSUM") as psO:

            oacc = [oaccp.tile([128, dg], FP, tag=f"oacc{sti}", name=f"oacc{sti}")
                    for sti in range(st_n)]
            agouts = []

            def attention_head(l):
                agin = dramp.tile([128, s], F16, tag=f"agin{l}", name=f"agin{l}")
                for qs in range(sq):
                    qsl = slice(512 * qs, 512 * qs + 512)
                    ps_av = psAV.tile([128, 512], FP, tag="ps_av", name="ps_av")
                    ps_dn = psDN.tile([1, 512], FP, tag="ps_dn", name="ps_dn")
                    nk = 4 * qs + 4
                    for kt in range(nk):
                        ps_s = psS.tile([128, 512], FP, tag="ps_s", name="ps_s")
                        nc.tensor.matmul(
                            ps_s[:],
                            kT[l][:, 128 * kt:128 * kt + 128],
                            qT[l][:, qsl],
                            start=True, stop=True,
                        )
                        et = etp.tile([128, 512], MM_DT, tag="et", name="et")
                        nc.scalar.activation(
                            et[:], ps_s[:],
                            mybir.ActivationFunctionType.Exp,
                            scale=scale,
                        )
                        off = 128 * kt - 512 * qs
                        if off >= 0:  # diagonal tile: apply causal mask
                            nc.vector.tensor_mul(
                                et[:], et[:], mask_sb[:, 384 - off:896 - off])
                        nc.tensor.matmul(
                            ps_dn[:], ones_col, et[:],
                            start=(kt == 0), stop=(kt == nk - 1),
                        )
                        nc.tensor.matmul(
                            ps_av[:],
                            vv[kt][:, 128 * l:128 * l + 128],
                            et[:],
                            start=(kt == 0), stop=(kt == nk - 1),
                        )
                    # normalize: aT[:, qsl] = ps_av * (1/denom) broadcast
                    dn_sb = rbp.tile([1, 512], FP, tag="dn_sb", name="dn_sb")
                    nc.vector.tensor_copy(dn_sb[:], ps_dn[:])
                    ps_rb = psDN.tile([128, 512], FP, tag="ps_rb", name="ps_rb", bufs=1)
                    nc.tensor.matmul(ps_rb[:], ones_row, dn_sb[:], start=True, stop=True)
                    rb_sb = rbp.tile([128, 512], FP, tag="rb_sb", name="rb_sb")
                    nc.vector.reciprocal(rb_sb[:], ps_rb[:])
                    an = atp.tile([128, 512], F16, tag="an", name="an")
                    nc.vector.tensor_mul(an[:], ps_av[:], rb_sb[:])
                    nc.sync.dma_start(agin[:, qsl], an[:])
                # AllGather this head's A^T across the batch group
                agout = dramp.tile([512, s], F16, tag=f"agout{l}", name=f"agout{l}")
                if ag:
                    nc.gpsimd.collective_compute(
                        "AllGather",
                        mybir.AluOpType.bypass,
                        replica_groups=REPLICA_GROUPS,
                        ins=[agin.opt()],
                        outs=[agout.opt()],
                    )
                else:  # timing ablation: local copy stands in for the collective
                    nc.sync.dma_start(agout[0:128, :], agin[:])
                agouts.append(agout)

            def outproj_pass(l):
                last = (l == nl - 1)
                wo4 = wop.tile([128, 4, dg], F16, tag="wo", name="wo", bufs=2)
                nc.sync.dma_start(
                    wo4[:],
                    wo_d[512 * l:512 * l + 512, :].rearrange("(c p) d -> p c d", p=128))
                for sti in range(st_n):
                    rs = slice(128 * sti, 128 * sti + 128)
                    at4 = atsp.tile([128, 4, 128], F16, tag="at4", name="at4")
                    nc.sync.dma_start(
                        at4[:],
                        agouts[l][:, rs].rearrange("(r p) t -> p r t", p=128))
                    ps_o = psO.tile([128, dg], FP, tag="ps_o", name="ps_o")
                    for r in range(4):
                        nc.tensor.matmul(
                            ps_o[:], at4[:, r, :], wo4[:, r, :],
                            start=(r == 0), stop=(r == 3),
                        )
                    if nl == 1:
                        ob = outp.tile([128, dg], FP, tag="ob", name="ob")
                        nc.vector.tensor_add(ob[:], ps_o[:], bo_sb[:])
                        nc.sync.dma_start(out_d[rs, :], ob[:])
                    elif l == 0:
                        nc.vector.tensor_add(oacc[sti][:], ps_o[:], bo_sb[:])
                    elif not last:
                        nc.vector.tensor_add(oacc[sti][:], ps_o[:], oacc[sti][:])
                    else:
                        ob = outp.tile([128, dg], FP, tag="ob", name="ob")
                        nc.vector.tensor_add(ob[:], ps_o[:], oacc[sti][:])
                        nc.sync.dma_start(out_d[rs, :], ob[:])

            for l in range(nl):
                attention_head(l)
                if l >= 1:
                    outproj_pass(l - 1)
            outproj_pass(nl - 1)


def make_inputs(x, W_qkv, b_qkv, W_out, b_out, s=S, h=H, nh=NH):
    """Host-side sharding: per-core input dicts."""
    nl = nh // GROUPS
    dg = nl * D
    x = np.ascontiguousarray(np.asarray(x, dtype=np.float32))
    W_qkv = np.asarray(W_qkv, dtype=np.float32)
    b_qkv = np.asarray(b_qkv, dtype=np.float32)
    W_out = np.asarray(W_out, dtype=np.float32)
    b_out = np.asarray(b_out, dtype=np.float32)

    # causal staircase master mask: mask[i, u] = 1 iff u >= i + 384
    uu = np.arange(896)[None, :]
    ii = np.arange(128)[:, None]
    mask = (uu >= ii + 384).astype(np.float32)
    ones = np.ones((128, 128), dtype=np.float32)

    WoT = W_out.T  # [h (d-in), h (n-out)]
    in_maps = []
    for c in range(NCORES):
        b, g = divmod(c, GROUPS)
        xT = np.ascontiguousarray(x[b].T)                       # [h, s]
        wq = np.ascontiguousarray(W_qkv[dg * g:dg * (g + 1), :].T)
        wk = np.ascontiguousarray(W_qkv[h + dg * g:h + dg * (g + 1), :].T)
        wv = np.ascontiguousarray(W_qkv[2 * h + dg * g:2 * h + dg * (g + 1), :].T)
        bq = np.ascontiguousarray(
            b_qkv[dg * g:dg * (g + 1)].reshape(nl, 128).T)      # [128, nl]
        bk = np.ascontiguousarray(
            b_qkv[h + dg * g:h + dg * (g + 1)].reshape(nl, 128).T)
        bv = np.tile(b_qkv[2 * h + dg * g:2 * h + dg * (g + 1)][None, :], (128, 1))
        bo = np.tile(b_out[dg * g:dg * (g + 1)][None, :], (128, 1))
        # W_out^T rows permuted to the AllGather d-order:
        # ci = l*4 + r  ->  global head 4r + l (within this batch group)
        blocks = []
        for l in range(nl):
            for r in range(GROUPS):
                hh = nl * r + l  # head held as local-head l by group-rank r
                blocks.append(WoT[D * hh:D * (hh + 1), dg * g:dg * (g + 1)])
        wo = np.ascontiguousarray(
            np.concatenate(blocks, axis=0).astype(np.float16))  # [h, dg] fp16
        in_maps.append({
            "xT": xT, "wq": wq, "wk": wk, "wv": wv, "wo": wo,
            "bq": bq, "bk": bk,
            "bv": np.ascontiguousarray(bv), "bo": np.ascontiguousarray(bo),
            "mask": mask, "ones": ones,
        })
    return in_maps


_NC_CACHE = {}


def _get_nc(key=(S, H, NH)):
    if key not in _NC_CACHE:
        _NC_CACHE[key] = build_nc(*key)
    return _NC_CACHE[key]


def kernel(x, W_qkv, b_qkv, W_out, b_out):
    global LAST_EXEC_NS, LAST_RESULTS
    nc = _get_nc()
    in_maps = make_inputs(x, W_qkv, b_qkv, W_out, b_out)
    res = run_bass_kernel_spmd(
        nc, in_maps, core_ids=list(range(NCORES)), trace=TRACE)
    LAST_EXEC_NS = res.exec_time_ns
    LAST_RESULTS = res
    nl = NH // GROUPS
    dg = nl * D
    out = np.empty((B, S, H), dtype=np.float32)
    for c in range(NCORES):
        b, g = divmod(c, GROUPS)
        out[b, :, dg * g:dg * (g + 1)] = res.results[c]["out"]
    return out



# revision 11
# speedup vs baseline: 1.1095x; 1.1095x over previous
"""Trainium2 Bass kernel: fused multi-head causal self-attention block.

Computes, for x:(B,S,H), W_qkv:(3H,H), b_qkv:(3H,), W_out:(H,H), b_out:(H,):
    qkv = x @ W_qkv.T + b_qkv ; split into q,k,v heads (NH heads, D=H/NH)
    out = softmax(causal(q k^T / sqrt(D))) v   ; merge heads
    return out @ W_out.T + b_out

Sharding over 8 NeuronCores: DP(2 batches) x TP(4 head-groups).
Core c handles batch b=c//4, head group g=c%4 (heads 4g..4g+3).

v2 design (single fused strip pipeline):
  - All matmul tensors bf16 (same PE rate as fp32r, half the DMA/SBUF,
    FWL-accelerated weight loads). PSUM accumulation stays fp32.
  - x is loaded once, host-prearranged per 512-column strip; Q^T/K^T and
    V projections run as PSUM-chained accumulations (2 banks total).
  - Attention runs strip-outer / head-inner; each strip's normalized A^T
    ([4*128, 512] f16) is AllGather'd across the 4-core batch group as
    soon as the strip finishes, so the output projection for strip s-1
    overlaps attention of strip s (interleaved at head granularity).
  - softmax denominator accumulated on the Vector engine (DVE) and
    contracted with a single ones-vector matmul per (head, strip);
    reciprocal is taken on the [1,512] row then broadcast via a 1-row
    matmul (cheap) instead of a [128,512] reciprocal (very slow).
  - Diagonal score tiles restrict the q-column range to the unmasked
    staircase, skipping fully-masked columns in scores/exp/AV.
Each core computes a disjoint 512-column slice of the output, so the
host does a pure concatenation.
"""

import math

import numpy as np
import ml_dtypes

import concourse.bass as bass
import concourse.mybir as mybir
import concourse.tile as tile
from concourse import bacc
from concourse.bass_utils import run_bass_kernel_spmd

FP = mybir.dt.float32
FR = mybir.dt.float32r
BF = mybir.dt.bfloat16
F16 = mybir.dt.float16

# Full-size problem constants.
B, S, H, NH = 2, 2048, 2048, 16
D = 128
NCORES = 8
GROUPS = 4                  # head-groups per batch (TP degree)
NL = NH // GROUPS           # local heads per core
DG = NL * D                 # per-core slice of the head dim
REPLICA_GROUPS = [[0, 1, 2, 3], [4, 5, 6, 7]]

TRACE = False               # set by test harness to capture NTFF profile
LAST_EXEC_NS = None
LAST_RESULTS = None


def build_nc(s=S, h=H, nh=NH, reps=1, ag=True):
    """Build the SPMD Bass program (identical on all 8 cores)."""
    nc = bacc.Bacc(
        "TRN2",
        target_bir_lowering=False,
        debug=False,
        enable_asserts=False,
        num_devices=NCORES,
    )

    nl = nh // GROUPS
    dg = nl * D
    hc = h // 128               # contraction chunks
    sq = s // 512               # 512-wide strips

    # ---- I/O (all host-prearranged for contiguous DMA) ----------------
    # x strips: [strip, 128, hc, 512] bf16 : x[st, p, c, t] = xT[128c+p, 512st+t]
    x_d = nc.dram_tensor("x", [sq, 128, hc, 512], BF, kind="ExternalInput")
    # weights: [128, hc, dg] : w[p, c, d] = W^T[128c+p, d]
    wq_d = nc.dram_tensor("wq", [128, hc, dg], BF, kind="ExternalInput")
    wk_d = nc.dram_tensor("wk", [128, hc, dg], BF, kind="ExternalInput")
    wv_d = nc.dram_tensor("wv", [128, hc, dg], BF, kind="ExternalInput")
    # out-proj weights, rows permuted to AG order: chunk c=(4r+l) <-> head 4r+l
    wo_d = nc.dram_tensor("wo", [128, hc, dg], F16, kind="ExternalInput")
    bq_d = nc.dram_tensor("bq", [128, nl], FP, kind="ExternalInput")
    bk_d = nc.dram_tensor("bk", [128, nl], FP, kind="ExternalInput")
    bv_d = nc.dram_tensor("bv", [128, dg], FP, kind="ExternalInput")
    bo_d = nc.dram_tensor("bo", [128, dg], FP, kind="ExternalInput")
    mask_d = nc.dram_tensor("mask", [128, 896], BF, kind="ExternalInput")
    ones_d = nc.dram_tensor("ones", [128, 128], FR, kind="ExternalInput")
    out_d = nc.dram_tensor("out", [s, dg], FP, kind="ExternalOutput")

    with tile.TileContext(nc) as tc:
        with tc.tile_pool(name="const", bufs=1) as constp:
            mask_sb = constp.tile([128, 896], BF)
            nc.sync.dma_start(mask_sb[:], mask_d[:])
            ones_sb = constp.tile([128, 128], FR)
            nc.sync.dma_start(ones_sb[:], ones_d[:])
            bq_sb = constp.tile([128, nl], FP)
            nc.sync.dma_start(bq_sb[:], bq_d[:])
            bk_sb = constp.tile([128, nl], FP)
            nc.sync.dma_start(bk_sb[:], bk_d[:])
            bv_sb = constp.tile([128, dg], FP)
            nc.sync.dma_start(bv_sb[:], bv_d[:])
            bo_sb = constp.tile([128, dg], FP)
            nc.sync.dma_start(bo_sb[:], bo_d[:])

            for _rep in range(reps):
                _emit_body(nc, tc, s, h, nh,
                           x_d, wq_d, wk_d, wv_d, wo_d, out_d,
                           bq_sb, bk_sb, bv_sb, bo_sb,
                           mask_sb, ones_sb, ag)

    nc.compile()
    return nc


def _emit_body(nc, tc, s, h, nh,
               x_d, wq_d, wk_d, wv_d, wo_d, out_d,
               bq_sb, bk_sb, bv_sb, bo_sb, mask_sb, ones_sb, ag=True):
    nl = nh // GROUPS
    dg = nl * D
    hc = h // 128
    sq = s // 512
    st_n = s // 128
    scale = 1.0 / math.sqrt(D)
    g_rank = None  # rank within replica group is implicit: host permutes wo

    ones_col = ones_sb[:, 0:1]   # [128,1] for denominator contract
    ones_row = ones_sb[0:1, :]   # [1,128] for partition broadcast

    with tc.tile_pool(name="wts", bufs=1) as wtp, \
         tc.tile_pool(name="xp", bufs=2) as xp, \
         tc.tile_pool(name="qkv", bufs=1) as qkvp, \
         tc.tile_pool(name="alocp", bufs=1) as alocp, \
         tc.tile_pool(name="atrp", bufs=1) as atrp, \
         tc.tile_pool(name="etp", bufs=3) as etp, \
         tc.tile_pool(name="dnp", bufs=2) as dnp, \
         tc.tile_pool(name="obp", bufs=2) as obp, \
         tc.tile_pool(name="dramp", bufs=1, space="DRAM") as dramp, \
         tc.tile_pool(name="psCH", bufs=2, space="PSUM") as psCH, \
         tc.tile_pool(name="psS", bufs=2, space="PSUM") as psS, \
         tc.tile_pool(name="psAV", bufs=2, space="PSUM") as psAV, \
         tc.tile_pool(name="psDR", bufs=1, space="PSUM") as psDR, \
         tc.tile_pool(name="psO", bufs=1, space="PSUM") as psO:

        # ---- persistent SBUF tensors -----------------------------------
        qT = [qkvp.tile([128, s], BF, tag=f"qT{t}", name=f"qT{t}") for t in range(nl)]
        kT = [qkvp.tile([128, s], BF, tag=f"kT{t}", name=f"kT{t}") for t in range(nl)]
        vv = [qkvp.tile([128, dg], BF, tag=f"v{t}", name=f"v{t}") for t in range(st_n)]
        aT = [alocp.tile([128, s], F16, tag=f"aT{t}", name=f"aT{t}") for t in range(nl)]

        # weights (whole-tensor contiguous DMAs)
        wq_sb = wtp.tile([128, hc, dg], BF, tag="wq", name="wq_sb")
        wk_sb = wtp.tile([128, hc, dg], BF, tag="wk", name="wk_sb")
        wv_sb = wtp.tile([128, hc, dg], BF, tag="wv", name="wv_sb")
        wo_sb = wtp.tile([128, hc, dg], F16, tag="wo", name="wo_sb")
        nc.sync.dma_start(wq_sb[:], wq_d[:])
        nc.sync.dma_start(wk_sb[:], wk_d[:])
        nc.sync.dma_start(wv_sb[:], wv_d[:])
        nc.sync.dma_start(wo_sb[:], wo_d[:])

        # AG buffers (DRAM)
        agin = [dramp.tile([4 * 128, 512], F16, tag=f"agin{st}", name=f"agin{st}")
                for st in range(sq)]
        agout = [dramp.tile([4 * 512, 512], F16, tag=f"agout{st}", name=f"agout{st}")
                 for st in range(sq)]

        def emit_qkv(strip):
            """Q^T,K^T,V projections for one 512-col strip of the sequence."""
            cs = slice(512 * strip, 512 * strip + 512)
            x_sb = xp.tile([128, hc, 512], BF, tag="xs", name="xs")
            # per-chunk sub-DMAs so the first chains can start early
            for c in range(hc):
                nc.sync.dma_start(x_sb[:, c, :], x_d[strip, :, c, :])
            # Q/K chains: one [128,512] psum accumulated over all hc chunks
            for gi in range(2 * nl):
                is_q = gi % 2 == 0          # interleave Q,K per head
                t = gi // 2
                w_sb = wq_sb if is_q else wk_sb
                ps = psCH.tile([128, 512], FP, tag="chain", name="ps_qk")
                for c in range(hc):
                    nc.tensor.matmul(
                        ps[:],
                        w_sb[:, c, 128 * t:128 * t + 128],
                        x_sb[:, c, :],
                        start=(c == 0), stop=(c == hc - 1),
                    )
                if is_q:
                    nc.scalar.activation(
                        qT[t][:, cs], ps[:],
                        mybir.ActivationFunctionType.Identity,
                        bias=bq_sb[:, t:t + 1],
                    )
                else:
                    nc.vector.tensor_scalar_add(kT[t][:, cs], ps[:], bk_sb[:, t:t + 1])
            # V chains: natural [s,d] layout, one per 128-row s-tile
            for sti in range(4):
                st_idx = 4 * strip + sti
                ps = psCH.tile([128, dg], FP, tag="chain", name="ps_v")
                for c in range(hc):
                    nc.tensor.matmul(
                        ps[:],
                        x_sb[:, c, 128 * sti:128 * sti + 128],
                        wv_sb[:, c, :],
                        start=(c == 0), stop=(c == hc - 1),
                    )
                nc.vector.tensor_add(vv[st_idx][:], ps[:], bv_sb[:])

        def emit_attention_head(strip, l):
            """Causal attention for head l restricted to q-strip `strip`."""
            qsl = slice(512 * strip, 512 * strip + 512)
            nk = 4 * strip + 4
            ps_av = psAV.tile([128, 512], FP, tag="ps_av", name="ps_av")
            dn_acc = dnp.tile([128, 512], FR, tag="dn_acc", name="dn_acc")
            ets = []
            # software-pipelined: scores(kt) ... AV(kt-1) so exp can run ahead
            for kt in range(nk + 1):
                if kt < nk:
                    c = kt - 4 * strip          # >=0 on diagonal tiles
                    qc = slice(128 * c, 512) if c >= 0 else slice(0, 512)
                    qg = slice(512 * strip + qc.start, 512 * strip + 512)
                    ps_s = psS.tile([128, 512], FP, tag="ps_s", name="ps_s")
                    nc.tensor.matmul(
                        ps_s[:, qc],
                        kT[l][:, 128 * kt:128 * kt + 128],
                        qT[l][:, qg],
                        start=True, stop=True,
                    )
                    et = etp.tile([128, 512], BF, tag="et", name="et")
                    nc.scalar.activation(
                        et[:, qc], ps_s[:, qc],
                        mybir.ActivationFunctionType.Exp,
                        scale=scale,
                    )
                    if c >= 0:
                        nc.vector.tensor_mul(
                            et[:, qc], et[:, qc], mask_sb[:, 384:896 - 128 * c])
                    # denominator accumulate on DVE
                    if kt == 0:
                        nc.vector.tensor_copy(dn_acc[:, qc], et[:, qc])
                    else:
                        nc.vector.tensor_add(dn_acc[:, qc], dn_acc[:, qc], et[:, qc])
                    ets.append((et, qc))
                if kt >= 1:
                    et, qc = ets[kt - 1]
                    nc.tensor.matmul(
                        ps_av[:, qc],
                        vv[kt - 1][:, 128 * l:128 * l + 128],
                        et[:, qc],
                        start=(kt - 1 == 0), stop=(kt - 1 == nk - 1),
                    )
            # denominator: contract partitions with ones, recip, broadcast
            ps_dn = psDR.tile([1, 512], FP, tag="dnrb", name="ps_dn")
            nc.tensor.matmul(ps_dn[:], ones_col, dn_acc[:],
                             start=True, stop=True)
            dn_sb = dnp.tile([1, 512], FP, tag="dn_sb", name="dn_sb")
            nc.vector.reciprocal(dn_sb[:], ps_dn[:])
            rb_sb = dnp.tile([128, 512], FP, tag="rb_sb", name="rb_sb")
            nc.gpsimd.partition_broadcast(rb_sb[:], dn_sb[:])
            nc.vector.tensor_mul(aT[l][:, qsl], ps_av[:], rb_sb[:])
            # ship this head's strip slice to the AG input buffer
            nc.sync.dma_start(agin[strip][128 * l:128 * l + 128, :], aT[l][:, qsl])

        def emit_ag(strip):
            if ag:
                nc.gpsimd.collective_compute(
                    "AllGather",
                    mybir.AluOpType.bypass,
                    replica_groups=REPLICA_GROUPS,
                    ins=[agin[strip].opt()],
                    outs=[agout[strip].opt()],
                )
            else:
                nc.sync.dma_start(agout[strip][0:512, :], agin[strip][:])

        at_r = {}

        def emit_atr_loads(strip):
            """Fetch the 3 remote rank blocks of agout[strip] into SBUF."""
            for r in range(GROUPS):
                t = atrp.tile([128, nl, 512], F16, tag=f"atr{r}", name=f"atr{r}")
                nc.sync.dma_start(
                    t[:],
                    agout[strip][512 * r:512 * r + 512, :]
                    .rearrange("(l p) t -> p l t", p=128))
                at_r[(strip, r)] = t

        def emit_outproj_sti(strip, sti):
            """One 128-row s-tile of the output projection for `strip`."""
            rs = slice(512 * strip + 128 * sti, 512 * strip + 128 * sti + 128)
            ssl = slice(128 * sti, 128 * sti + 128)
            ps_o = psO.tile([128, dg], FP, tag="ps_o", name="ps_o")
            for c in range(hc):
                r, l = divmod(c, nl)
                lhsT = at_r[(strip, r)][:, l, ssl]
                nc.tensor.matmul(
                    ps_o[:], lhsT, wo_sb[:, c, :],
                    start=(c == 0), stop=(c == hc - 1),
                )
            ob = obp.tile([128, dg], FP, tag="ob", name="ob")
            nc.vector.tensor_add(ob[:], ps_o[:], bo_sb[:])
            nc.sync.dma_start(out_d[rs, :], ob[:])

        # ---- main fused pipeline ---------------------------------------
        for strip in range(sq):
            emit_qkv(strip)
            if strip >= 1:
                emit_atr_loads(strip - 1)
            for l in range(nl):
                emit_attention_head(strip, l)
                if strip >= 1:
                    emit_outproj_sti(strip - 1, l)
            emit_ag(strip)
        emit_atr_loads(sq - 1)
        for sti in range(4):
            emit_outproj_sti(sq - 1, sti)


def make_inputs(x, W_qkv, b_qkv, W_out, b_out, s=S, h=H, nh=NH):
    """Host-side sharding: per-core input dicts (layout prep only)."""
    nl = nh // GROUPS
    dg = nl * D
    hc = h // 128
    sq = s // 512
    bf16 = ml_dtypes.bfloat16
    x = np.asarray(x, dtype=np.float32)
    W_qkv = np.asarray(W_qkv, dtype=np.float32)
    b_qkv = np.asarray(b_qkv, dtype=np.float32)
    W_out = np.asarray(W_out, dtype=np.float32)
    b_out = np.asarray(b_out, dtype=np.float32)

    # causal staircase master mask: mask[i, u] = 1 iff u >= i + 384
    uu = np.arange(896)[None, :]
    ii = np.arange(128)[:, None]
    mask = (uu >= ii + 384).astype(bf16)
    ones = np.ones((128, 128), dtype=np.float32)

    WoT = W_out.T  # [h (d-in), h (n-out)]
    in_maps = []
    for core in range(NCORES):
        b, g = divmod(core, GROUPS)
        xT = x[b].T                                   # [h, s]
        # x strips: [sq, 128, hc, 512]
        xs = np.ascontiguousarray(
            xT.reshape(hc, 128, sq, 512).transpose(2, 1, 0, 3).astype(bf16))

        def arr_w(wslice, dt):
            # [dg, h] -> transposed chunks [128, hc, dg]
            return np.ascontiguousarray(
                wslice.T.reshape(hc, 128, dg).transpose(1, 0, 2).astype(dt))

        wq = arr_w(W_qkv[dg * g:dg * (g + 1), :], bf16)
        wk = arr_w(W_qkv[h + dg * g:h + dg * (g + 1), :], bf16)
        wv = arr_w(W_qkv[2 * h + dg * g:2 * h + dg * (g + 1), :], bf16)
        bq = np.ascontiguousarray(
            b_qkv[dg * g:dg * (g + 1)].reshape(nl, 128).T)      # [128, nl]
        bk = np.ascontiguousarray(
            b_qkv[h + dg * g:h + dg * (g + 1)].reshape(nl, 128).T)
        bv = np.tile(b_qkv[2 * h + dg * g:2 * h + dg * (g + 1)][None, :], (128, 1))
        bo = np.tile(b_out[dg * g:dg * (g + 1)][None, :], (128, 1))
        # W_out^T rows permuted to the AG d-order: chunk c = 4r+l -> head 4r+l
        blocks = []
        for r in range(GROUPS):
            for l in range(nl):
                hh = nl * r + l
                blocks.append(WoT[D * hh:D * (hh + 1), dg * g:dg * (g + 1)])
        wo = np.ascontiguousarray(
            np.concatenate(blocks, axis=0)
            .reshape(hc, 128, dg).transpose(1, 0, 2).astype(np.float16))
        in_maps.append({
            "x": xs, "wq": wq, "wk": wk, "wv": wv, "wo": wo,
            "bq": bq, "bk": bk,
            "bv": np.ascontiguousarray(bv.astype(np.float32)),
            "bo": np.ascontiguousarray(bo.astype(np.float32)),
            "mask": mask, "ones": ones,
        })
    return in_maps


_NC_CACHE = {}


def _get_nc(key=(S, H, NH)):
    if key not in _NC_CACHE:
        _NC_CACHE[key] = build_nc(*key)
    return _NC_CACHE[key]


def kernel(x, W_qkv, b_qkv, W_out, b_out):
    global LAST_EXEC_NS, LAST_RESULTS
    nc = _get_nc()
    in_maps = make_inputs(x, W_qkv, b_qkv, W_out, b_out)
    res = run_bass_kernel_spmd(
        nc, in_maps, core_ids=list(range(NCORES)), trace=TRACE)
    LAST_EXEC_NS = res.exec_time_ns
    LAST_RESULTS = res
    out = np.empty((B, S, H), dtype=np.float32)
    for core in range(NCORES):
        b, g = divmod(core, GROUPS)
        out[b, :, DG * g:DG * (g + 1)] = res.results[core]["out"]
    return out


# revision 18
# speedup vs baseline: 1.2581x; 1.1340x over previous
"""Trainium2 Bass kernel: fused multi-head causal self-attention block.

Computes, for x:(B,S,H), W_qkv:(3H,H), b_qkv:(3H,), W_out:(H,H), b_out:(H,):
    qkv = x @ W_qkv.T + b_qkv ; split into q,k,v heads (NH heads, D=H/NH)
    out = softmax(causal(q k^T / sqrt(D))) v   ; merge heads
    return out @ W_out.T + b_out

Sharding over 8 NeuronCores: DP(2 batches) x TP(4 head-groups).
Core c handles batch b=c//4, head group g=c%4 (heads 4g..4g+3).

v2 design (single fused strip pipeline):
  - All matmul tensors bf16 (same PE rate as fp32r, half the DMA/SBUF,
    FWL-accelerated weight loads). PSUM accumulation stays fp32.
  - x is loaded once, host-prearranged per 512-column strip; Q^T/K^T and
    V projections run as PSUM-chained accumulations (2 banks total).
  - Attention runs strip-outer / head-inner; each strip's normalized A^T
    ([4*128, 512] f16) is AllGather'd across the 4-core batch group as
    soon as the strip finishes, so the output projection for strip s-1
    overlaps attention of strip s (interleaved at head granularity).
  - softmax denominator accumulated on the Vector engine (DVE) and
    contracted with a single ones-vector matmul per (head, strip);
    reciprocal is taken on the [1,512] row then broadcast via a 1-row
    matmul (cheap) instead of a [128,512] reciprocal (very slow).
  - Diagonal score tiles restrict the q-column range to the unmasked
    staircase, skipping fully-masked columns in scores/exp/AV.
Each core computes a disjoint 512-column slice of the output, so the
host does a pure concatenation.
"""

import math

import numpy as np
import ml_dtypes

import concourse.bass as bass
import concourse.mybir as mybir
import concourse.tile as tile
from concourse import bacc, bass_isa
from concourse.bass_utils import run_bass_kernel_spmd

FP = mybir.dt.float32
FR = mybir.dt.float32r
BF = mybir.dt.bfloat16
F16 = mybir.dt.float16

# Full-size problem constants.
B, S, H, NH = 2, 2048, 2048, 16
D = 128
NCORES = 8
GROUPS = 4                  # head-groups per batch (TP degree)
NL = NH // GROUPS           # local heads per core
DG = NL * D                 # per-core slice of the head dim
REPLICA_GROUPS = [[0, 1, 2, 3], [4, 5, 6, 7]]

TRACE = False               # set by test harness to capture NTFF profile
LAST_EXEC_NS = None
LAST_RESULTS = None


def build_nc(s=S, h=H, nh=NH, reps=1, ag=True):
    """Build the SPMD Bass program (identical on all 8 cores)."""
    nc = bacc.Bacc(
        "TRN2",
        target_bir_lowering=False,
        debug=False,
        enable_asserts=False,
        num_devices=NCORES,
    )

    nl = nh // GROUPS
    dg = nl * D
    hc = h // 128               # contraction chunks
    sq = s // 512               # 512-wide strips

    # ---- I/O (all host-prearranged for contiguous DMA) ----------------
    # x strips: [strip, 128, hc, 512] bf16 : x[st, p, c, t] = xT[128c+p, 512st+t]
    x_d = nc.dram_tensor("x", [sq, 128, hc, 512], BF, kind="ExternalInput")
    # weights: [128, hc, dg] : w[p, c, d] = W^T[128c+p, d]
    wq_d = nc.dram_tensor("wq", [128, hc, dg], BF, kind="ExternalInput")
    wk_d = nc.dram_tensor("wk", [128, hc, dg], BF, kind="ExternalInput")
    wv_d = nc.dram_tensor("wv", [128, hc, dg], BF, kind="ExternalInput")
    # out-proj weights, rows permuted to AG order: chunk c=(4r+l) <-> head 4r+l
    wo_d = nc.dram_tensor("wo", [128, hc, dg], F16, kind="ExternalInput")
    bq_d = nc.dram_tensor("bq", [128, nl], FP, kind="ExternalInput")
    bk_d = nc.dram_tensor("bk", [128, nl], FP, kind="ExternalInput")
    bv_d = nc.dram_tensor("bv", [128, dg], FP, kind="ExternalInput")
    bo_d = nc.dram_tensor("bo", [128, dg], FP, kind="ExternalInput")
    mask_d = nc.dram_tensor("mask", [128, 896], BF, kind="ExternalInput")
    out_d = nc.dram_tensor("out", [s, dg], FP, kind="ExternalOutput")

    with tile.TileContext(nc) as tc:
        with tc.tile_pool(name="const", bufs=1) as constp:
            mask_sb = constp.tile([128, 896], BF)
            nc.sync.dma_start(mask_sb[:], mask_d[:])
            bq_sb = constp.tile([128, nl], FP)
            nc.sync.dma_start(bq_sb[:], bq_d[:])
            bk_sb = constp.tile([128, nl], FP)
            nc.sync.dma_start(bk_sb[:], bk_d[:])
            bv_sb = constp.tile([128, dg], FP)
            nc.sync.dma_start(bv_sb[:], bv_d[:])
            bo_sb = constp.tile([128, dg], FP)
            nc.sync.dma_start(bo_sb[:], bo_d[:])

            for _rep in range(reps):
                _emit_body(nc, tc, s, h, nh,
                           x_d, wq_d, wk_d, wv_d, wo_d, out_d,
                           bq_sb, bk_sb, bv_sb, bo_sb,
                           mask_sb, ag)

    nc.compile()
    return nc


def _emit_body(nc, tc, s, h, nh,
               x_d, wq_d, wk_d, wv_d, wo_d, out_d,
               bq_sb, bk_sb, bv_sb, bo_sb, mask_sb, ag=True):
    nl = nh // GROUPS
    dg = nl * D
    hc = h // 128
    sq = s // 512
    st_n = s // 128
    scale = 1.0 / math.sqrt(D)

    with tc.tile_pool(name="wts", bufs=1) as wtp, \
         tc.tile_pool(name="xp", bufs=2) as xp, \
         tc.tile_pool(name="qkv", bufs=1) as qkvp, \
         tc.tile_pool(name="atrp", bufs=2) as atrp, \
         tc.tile_pool(name="etp", bufs=3) as etp, \
         tc.tile_pool(name="anp", bufs=2) as anp, \
         tc.tile_pool(name="dnp", bufs=2) as dnp, \
         tc.tile_pool(name="obp", bufs=2) as obp, \
         tc.tile_pool(name="dramp", bufs=1, space="DRAM") as dramp, \
         tc.tile_pool(name="psCH", bufs=2, space="PSUM") as psCH, \
         tc.tile_pool(name="psS", bufs=2, space="PSUM") as psS, \
         tc.tile_pool(name="psAV", bufs=2, space="PSUM") as psAV, \
         tc.tile_pool(name="psO", bufs=2, space="PSUM") as psO:

        # ---- persistent SBUF tensors -----------------------------------
        qT = [qkvp.tile([128, s], BF, tag=f"qT{t}", name=f"qT{t}") for t in range(nl)]
        kT = [qkvp.tile([128, s], BF, tag=f"kT{t}", name=f"kT{t}") for t in range(nl)]
        vv = [qkvp.tile([128, dg], BF, tag=f"v{t}", name=f"v{t}") for t in range(st_n)]

        x_sbs = {}

        def emit_x_load(strip):
            x_sb = xp.tile([128, hc, 512], BF, tag="xs", name="xs")
            # per-chunk sub-DMAs so the first chains can start early
            for c in range(hc):
                nc.sync.dma_start(x_sb[:, c, :], x_d[strip, :, c, :])
            x_sbs[strip] = x_sb

        # first strip of x before anything else, then weights
        emit_x_load(0)
        wq_sb = wtp.tile([128, hc, dg], BF, tag="wq", name="wq_sb")
        wk_sb = wtp.tile([128, hc, dg], BF, tag="wk", name="wk_sb")
        wv_sb = wtp.tile([128, hc, dg], BF, tag="wv", name="wv_sb")
        wo_sb = wtp.tile([128, hc, dg], F16, tag="wo", name="wo_sb")
        nc.sync.dma_start(wq_sb[:], wq_d[:])
        nc.sync.dma_start(wk_sb[:], wk_d[:])
        nc.sync.dma_start(wv_sb[:], wv_d[:])
        nc.sync.dma_start(wo_sb[:], wo_d[:])

        # AG buffers (DRAM)
        agin = [dramp.tile([4 * 128, 512], F16, tag=f"agin{st}", name=f"agin{st}")
                for st in range(sq)]
        agout = [dramp.tile([4 * 512, 512], F16, tag=f"agout{st}", name=f"agout{st}")
                 for st in range(sq)]

        def emit_qkv(strip):
            """Q^T,K^T,V projections for one 512-col strip of the sequence."""
            cs = slice(512 * strip, 512 * strip + 512)
            x_sb = x_sbs.pop(strip)
            # Q/K chains: one [128,512] psum accumulated over all hc chunks
            for gi in range(2 * nl):
                is_q = gi % 2 == 0          # interleave Q,K per head
                t = gi // 2
                w_sb = wq_sb if is_q else wk_sb
                ps = psCH.tile([128, 512], FP, tag="chain", name="ps_qk")
                for c in range(hc):
                    nc.tensor.matmul(
                        ps[:],
                        w_sb[:, c, 128 * t:128 * t + 128],
                        x_sb[:, c, :],
                        start=(c == 0), stop=(c == hc - 1),
                    )
                if is_q:
                    nc.scalar.activation(
                        qT[t][:, cs], ps[:],
                        mybir.ActivationFunctionType.Identity,
                        bias=bq_sb[:, t:t + 1],
                    )
                else:
                    nc.vector.tensor_scalar_add(kT[t][:, cs], ps[:], bk_sb[:, t:t + 1])
            # V chains: natural [s,d] layout, one per 128-row s-tile
            for sti in range(4):
                st_idx = 4 * strip + sti
                ps = psCH.tile([128, dg], FP, tag="chain", name="ps_v")
                for c in range(hc):
                    nc.tensor.matmul(
                        ps[:],
                        x_sb[:, c, 128 * sti:128 * sti + 128],
                        wv_sb[:, c, :],
                        start=(c == 0), stop=(c == hc - 1),
                    )
                nc.vector.tensor_add(vv[st_idx][:], ps[:], bv_sb[:])

        def emit_attention_head(strip, l):
            """Causal attention for head l restricted to q-strip `strip`."""
            nk = 4 * strip + 4
            ps_av = psAV.tile([128, 512], FP, tag="ps_av", name="ps_av")
            dn_acc = dnp.tile([128, 512], FP, tag="dn_acc", name="dn_acc")
            ets = []
            # software-pipelined: scores(kt) ... AV(kt-1) so exp can run ahead
            for kt in range(nk + 1):
                if kt < nk:
                    c = kt - 4 * strip          # >=0 on diagonal tiles
                    qc = slice(128 * c, 512) if c >= 0 else slice(0, 512)
                    qg = slice(512 * strip + qc.start, 512 * strip + 512)
                    ps_s = psS.tile([128, 512], FP, tag="ps_s", name="ps_s")
                    nc.tensor.matmul(
                        ps_s[:, qc],
                        kT[l][:, 128 * kt:128 * kt + 128],
                        qT[l][:, qg],
                        start=True, stop=True,
                    )
                    et = etp.tile([128, 512], BF, tag="et", name="et")
                    nc.scalar.activation(
                        et[:, qc], ps_s[:, qc],
                        mybir.ActivationFunctionType.Exp,
                        scale=scale,
                    )
                    if c >= 0:
                        nc.vector.tensor_mul(
                            et[:, qc], et[:, qc], mask_sb[:, 384:896 - 128 * c])
                    # denominator accumulate on DVE
                    if kt == 0:
                        nc.vector.tensor_copy(dn_acc[:, qc], et[:, qc])
                    else:
                        nc.vector.tensor_add(dn_acc[:, qc], dn_acc[:, qc], et[:, qc])
                    ets.append((et, qc))
                if kt >= 1:
                    et, qc = ets[kt - 1]
                    nc.tensor.matmul(
                        ps_av[:, qc],
                        vv[kt - 1][:, 128 * l:128 * l + 128],
                        et[:, qc],
                        start=(kt - 1 == 0), stop=(kt - 1 == nk - 1),
                    )
            # denominator: all-reduce over partitions (sum broadcast to all
            # 128 partitions), then fast approx reciprocal, then normalize.
            dnsum = dnp.tile([128, 512], FP, tag="dnsum", name="dnsum")
            nc.gpsimd.partition_all_reduce(
                dnsum[:], dn_acc[:], channels=128, reduce_op=bass_isa.ReduceOp.add)
            rb_sb = dnp.tile([128, 512], FP, tag="rb_sb", name="rb_sb")
            nc.vector.reciprocal_approx_fast(rb_sb[:], dnsum[:])
            an = anp.tile([128, 512], F16, tag="an", name="an")
            nc.vector.tensor_mul(an[:], ps_av[:], rb_sb[:])
            # ship this head's strip slice to the AG input buffer
            nc.gpsimd.dma_start(agin[strip][128 * l:128 * l + 128, :], an[:])

        def emit_ag(strip):
            if ag:
                nc.gpsimd.collective_compute(
                    "AllGather",
                    mybir.AluOpType.bypass,
                    replica_groups=REPLICA_GROUPS,
                    ins=[agin[strip].opt()],
                    outs=[agout[strip].opt()],
                )
            else:
                nc.gpsimd.dma_start(agout[strip][0:512, :], agin[strip][:])

        at_r = {}

        def emit_atr_loads(strip):
            """Fetch the 4 rank blocks of agout[strip] into SBUF."""
            for r in range(GROUPS):
                t = atrp.tile([128, nl, 512], F16, tag=f"atr{r}", name=f"atr{r}")
                nc.scalar.dma_start(
                    t[:],
                    agout[strip][512 * r:512 * r + 512, :]
                    .rearrange("(l p) t -> p l t", p=128))
                at_r[(strip, r)] = t

        def emit_outproj_sti(strip, sti):
            """One 128-row s-tile of the output projection for `strip`."""
            rs = slice(512 * strip + 128 * sti, 512 * strip + 128 * sti + 128)
            ssl = slice(128 * sti, 128 * sti + 128)
            ps_o = psO.tile([128, dg], FP, tag="ps_o", name="ps_o")
            for c in range(hc):
                r, l = divmod(c, nl)
                lhsT = at_r[(strip, r)][:, l, ssl]
                nc.tensor.matmul(
                    ps_o[:], lhsT, wo_sb[:, c, :],
                    start=(c == 0), stop=(c == hc - 1),
                )
            ob = obp.tile([128, dg], FP, tag="ob", name="ob")
            nc.vector.tensor_add(ob[:], ps_o[:], bo_sb[:])
            nc.sync.dma_start(out_d[rs, :], ob[:])

        # ---- main fused pipeline ---------------------------------------
        # outproj for strip s runs during strip s+2 so even a slow AllGather
        # never head-of-line-blocks the PE queue.
        for strip in range(sq):
            if strip + 1 < sq:
                emit_x_load(strip + 1)
            emit_qkv(strip)
            if strip >= 2:
                emit_atr_loads(strip - 2)
            for l in range(nl):
                emit_attention_head(strip, l)
                if strip >= 2:
                    emit_outproj_sti(strip - 2, l)
            emit_ag(strip)
        for strip in (sq - 2, sq - 1):
            emit_atr_loads(strip)
            for sti in range(4):
                emit_outproj_sti(strip, sti)


def make_inputs(x, W_qkv, b_qkv, W_out, b_out, s=S, h=H, nh=NH):
    """Host-side sharding: per-core input dicts (layout prep only)."""
    nl = nh // GROUPS
    dg = nl * D
    hc = h // 128
    sq = s // 512
    bf16 = ml_dtypes.bfloat16
    x = np.asarray(x, dtype=np.float32)
    W_qkv = np.asarray(W_qkv, dtype=np.float32)
    b_qkv = np.asarray(b_qkv, dtype=np.float32)
    W_out = np.asarray(W_out, dtype=np.float32)
    b_out = np.asarray(b_out, dtype=np.float32)

    # causal staircase master mask: mask[i, u] = 1 iff u >= i + 384
    uu = np.arange(896)[None, :]
    ii = np.arange(128)[:, None]
    mask = (uu >= ii + 384).astype(bf16)

    WoT = W_out.T  # [h (d-in), h (n-out)]
    in_maps = []
    for core in range(NCORES):
        b, g = divmod(core, GROUPS)
        xT = x[b].T                                   # [h, s]
        # x strips: [sq, 128, hc, 512]
        xs = np.ascontiguousarray(
            xT.reshape(hc, 128, sq, 512).transpose(2, 1, 0, 3).astype(bf16))

        def arr_w(wslice, dt):
            # [dg, h] -> transposed chunks [128, hc, dg]
            return np.ascontiguousarray(
                wslice.T.reshape(hc, 128, dg).transpose(1, 0, 2).astype(dt))

        wq = arr_w(W_qkv[dg * g:dg * (g + 1), :], bf16)
        wk = arr_w(W_qkv[h + dg * g:h + dg * (g + 1), :], bf16)
        wv = arr_w(W_qkv[2 * h + dg * g:2 * h + dg * (g + 1), :], bf16)
        bq = np.ascontiguousarray(
            b_qkv[dg * g:dg * (g + 1)].reshape(nl, 128).T)      # [128, nl]
        bk = np.ascontiguousarray(
            b_qkv[h + dg * g:h + dg * (g + 1)].reshape(nl, 128).T)
        bv = np.tile(b_qkv[2 * h + dg * g:2 * h + dg * (g + 1)][None, :], (128, 1))
        bo = np.tile(b_out[dg * g:dg * (g + 1)][None, :], (128, 1))
        # W_out^T rows permuted to the AG d-order: chunk c = 4r+l -> head 4r+l
        blocks = []
        for r in range(GROUPS):
            for l in range(nl):
                hh = nl * r + l
                blocks.append(WoT[D * hh:D * (hh + 1), dg * g:dg * (g + 1)])
        wo = np.ascontiguousarray(
            np.concatenate(blocks, axis=0)
            .reshape(hc, 128, dg).transpose(1, 0, 2).astype(np.float16))
        in_maps.append({
            "x": xs, "wq": wq, "wk": wk, "wv": wv, "wo": wo,
            "bq": bq, "bk": bk,
            "bv": np.ascontiguousarray(bv.astype(np.float32)),
            "bo": np.ascontiguousarray(bo.astype(np.float32)),
            "mask": mask,
        })
    return in_maps


_NC_CACHE = {}


def _get_nc(key=(S, H, NH)):
    if key not in _NC_CACHE:
        _NC_CACHE[key] = build_nc(*key)
    return _NC_CACHE[key]


def kernel(x, W_qkv, b_qkv, W_out, b_out):
    global LAST_EXEC_NS, LAST_RESULTS
    nc = _get_nc()
    in_maps = make_inputs(x, W_qkv, b_qkv, W_out, b_out)
    res = run_bass_kernel_spmd(
        nc, in_maps, core_ids=list(range(NCORES)), trace=TRACE)
    LAST_EXEC_NS = res.exec_time_ns
    LAST_RESULTS = res
    out = np.empty((B, S, H), dtype=np.float32)
    for core in range(NCORES):
        b, g = divmod(core, GROUPS)
        out[b, :, DG * g:DG * (g + 1)] = res.results[core]["out"]
    return out


# revision 31
# speedup vs baseline: 1.3153x; 1.0454x over previous
"""Trainium2 Bass kernel: fused multi-head causal self-attention block.

Computes, for x:(B,S,H), W_qkv:(3H,H), b_qkv:(3H,), W_out:(H,H), b_out:(H,):
    qkv = x @ W_qkv.T + b_qkv ; split into q,k,v heads (NH heads, D=H/NH)
    out = softmax(causal(q k^T / sqrt(D))) v   ; merge heads
    return out @ W_out.T + b_out

Sharding over 8 NeuronCores: DP(2 batches) x TP(4 head-groups).
Core c handles batch b=c//4, head group g=c%4 (heads 4g..4g+3).

v2 design (single fused strip pipeline):
  - All matmul tensors bf16 (same PE rate as fp32r, half the DMA/SBUF,
    FWL-accelerated weight loads). PSUM accumulation stays fp32.
  - x is loaded once, host-prearranged per 512-column strip; Q^T/K^T and
    V projections run as PSUM-chained accumulations (2 banks total).
  - Attention runs strip-outer / head-inner; each strip's normalized A^T
    ([4*128, 512] f16) is AllGather'd across the 4-core batch group as
    soon as the strip finishes, so the output projection for strip s-1
    overlaps attention of strip s (interleaved at head granularity).
  - softmax denominator accumulated on the Vector engine (DVE) and
    contracted with a single ones-vector matmul per (head, strip);
    reciprocal is taken on the [1,512] row then broadcast via a 1-row
    matmul (cheap) instead of a [128,512] reciprocal (very slow).
  - Diagonal score tiles restrict the q-column range to the unmasked
    staircase, skipping fully-masked columns in scores/exp/AV.
Each core computes a disjoint 512-column slice of the output, so the
host does a pure concatenation.
"""

import math

import numpy as np
import ml_dtypes

import concourse.bass as bass
import concourse.mybir as mybir
import concourse.tile as tile
from concourse import bacc, bass_isa
from concourse.bass_utils import run_bass_kernel_spmd

FP = mybir.dt.float32
FR = mybir.dt.float32r
BF = mybir.dt.bfloat16
F16 = mybir.dt.float16

# Full-size problem constants.
B, S, H, NH = 2, 2048, 2048, 16
D = 128
NCORES = 8
GROUPS = 4                  # head-groups per batch (TP degree)
NL = NH // GROUPS           # local heads per core
DG = NL * D                 # per-core slice of the head dim
REPLICA_GROUPS = [[0, 1, 2, 3], [4, 5, 6, 7]]

TRACE = False               # set by test harness to capture NTFF profile
LAST_EXEC_NS = None
LAST_RESULTS = None


def build_nc(s=S, h=H, nh=NH, reps=1, ag=True):
    """Build the SPMD Bass program (identical on all 8 cores)."""
    nc = bacc.Bacc(
        "TRN2",
        target_bir_lowering=False,
        debug=False,
        enable_asserts=False,
        num_devices=NCORES,
    )

    nl = nh // GROUPS
    dg = nl * D
    hc = h // 128               # contraction chunks
    sq = s // 512               # 512-wide strips

    # ---- I/O (all host-prearranged for contiguous DMA) ----------------
    # x strips: [strip, 128, hc, 512] bf16 : x[st, p, c, t] = xT[128c+p, 512st+t]
    x_d = nc.dram_tensor("x", [sq, 128, hc, 512], BF, kind="ExternalInput")
    # weights: [128, hc, dg] : w[p, c, d] = W^T[128c+p, d]
    wq_d = nc.dram_tensor("wq", [128, hc, dg], BF, kind="ExternalInput")
    wk_d = nc.dram_tensor("wk", [128, hc, dg], BF, kind="ExternalInput")
    wv_d = nc.dram_tensor("wv", [128, hc, dg], BF, kind="ExternalInput")
    # out-proj weights, rows permuted to AG order: chunk c=(4r+l) <-> head 4r+l
    wo_d = nc.dram_tensor("wo", [128, hc, dg], F16, kind="ExternalInput")
    bq_d = nc.dram_tensor("bq", [128, nl], FP, kind="ExternalInput")
    bk_d = nc.dram_tensor("bk", [128, nl], FP, kind="ExternalInput")
    bv_d = nc.dram_tensor("bv", [128, dg], FP, kind="ExternalInput")
    bo_d = nc.dram_tensor("bo", [128, dg], FP, kind="ExternalInput")
    mask_d = nc.dram_tensor("mask", [128, 896], BF, kind="ExternalInput")
    ones_d = nc.dram_tensor("ones", [128, 128], FR, kind="ExternalInput")
    out_d = nc.dram_tensor("out", [s, dg], FP, kind="ExternalOutput")

    with tile.TileContext(nc) as tc:
        with tc.tile_pool(name="const", bufs=1) as constp:
            mask_sb = constp.tile([128, 896], BF)
            nc.sync.dma_start(mask_sb[:], mask_d[:])
            ones_sb = constp.tile([128, 128], FR)
            nc.sync.dma_start(ones_sb[:], ones_d[:])
            bq_sb = constp.tile([128, nl], FP)
            nc.sync.dma_start(bq_sb[:], bq_d[:])
            bk_sb = constp.tile([128, nl], FP)
            nc.sync.dma_start(bk_sb[:], bk_d[:])
            bv_sb = constp.tile([128, dg], FP)
            nc.sync.dma_start(bv_sb[:], bv_d[:])
            bo_sb = constp.tile([128, dg], FP)
            nc.sync.dma_start(bo_sb[:], bo_d[:])

            for _rep in range(reps):
                _emit_body(nc, tc, s, h, nh,
                           x_d, wq_d, wk_d, wv_d, wo_d, out_d,
                           bq_sb, bk_sb, bv_sb, bo_sb,
                           mask_sb, ones_sb, ag)

    nc.compile()
    return nc


def _emit_body(nc, tc, s, h, nh,
               x_d, wq_d, wk_d, wv_d, wo_d, out_d,
               bq_sb, bk_sb, bv_sb, bo_sb, mask_sb, ones_sb, ag=True):
    nl = nh // GROUPS
    dg = nl * D
    hc = h // 128
    sq = s // 512
    st_n = s // 128
    scale = 1.0 / math.sqrt(D)
    ones_col = ones_sb[:, 0:1]   # [128,1] FR: denominator partition-contract
    ones_row = ones_sb[0:1, :]   # [1,128] FR: partition broadcast

    with tc.tile_pool(name="wts", bufs=1) as wtp, \
         tc.tile_pool(name="xp", bufs=2) as xp, \
         tc.tile_pool(name="qkv", bufs=1) as qkvp, \
         tc.tile_pool(name="atrp", bufs=2) as atrp, \
         tc.tile_pool(name="etp", bufs=3) as etp, \
         tc.tile_pool(name="anp", bufs=2) as anp, \
         tc.tile_pool(name="dnp", bufs=2) as dnp, \
         tc.tile_pool(name="obp", bufs=1) as obp, \
         tc.tile_pool(name="dramp", bufs=1, space="DRAM") as dramp, \
         tc.tile_pool(name="psCH", bufs=2, space="PSUM") as psCH, \
         tc.tile_pool(name="psS", bufs=2, space="PSUM") as psS, \
         tc.tile_pool(name="psAV", bufs=2, space="PSUM") as psAV, \
         tc.tile_pool(name="psDR", bufs=1, space="PSUM") as psDR, \
         tc.tile_pool(name="psO", bufs=1, space="PSUM") as psO:

        # ---- persistent SBUF tensors -----------------------------------
        qT = [qkvp.tile([128, s], BF, tag=f"qT{t}", name=f"qT{t}") for t in range(nl)]
        kT = [qkvp.tile([128, s], BF, tag=f"kT{t}", name=f"kT{t}") for t in range(nl)]
        vv = [qkvp.tile([128, dg], BF, tag=f"v{t}", name=f"v{t}") for t in range(st_n)]

        x_sbs = {}

        def emit_x_load(strip):
            x_sb = xp.tile([128, hc, 512], BF, tag="xs", name="xs")
            # per-chunk sub-DMAs so the first chains can start early
            for c in range(hc):
                nc.sync.dma_start(x_sb[:, c, :], x_d[strip, :, c, :])
            x_sbs[strip] = x_sb

        # first QK weights, then strip-0 x, then the rest of the weights
        wq_sb = wtp.tile([128, hc, dg], BF, tag="wq", name="wq_sb")
        wk_sb = wtp.tile([128, hc, dg], BF, tag="wk", name="wk_sb")
        wv_sb = wtp.tile([128, hc, dg], BF, tag="wv", name="wv_sb")
        wo_sb = wtp.tile([128, hc, dg], F16, tag="wo", name="wo_sb")
        nc.sync.dma_start(wq_sb[:], wq_d[:])
        emit_x_load(0)
        nc.sync.dma_start(wk_sb[:], wk_d[:])
        nc.scalar.dma_start(wv_sb[:], wv_d[:])
        nc.scalar.dma_start(wo_sb[:], wo_d[:])

        # AG buffers (DRAM)
        agin = [dramp.tile([4 * 128, 512], F16, tag=f"agin{st}", name=f"agin{st}")
                for st in range(sq)]
        agout = [dramp.tile([4 * 512, 512], F16, tag=f"agout{st}", name=f"agout{st}")
                 for st in range(sq)]

        def emit_qkv(strip):
            """Q^T,K^T,V projections for one 512-col strip of the sequence."""
            cs = slice(512 * strip, 512 * strip + 512)
            x_sb = x_sbs.pop(strip)
            # Q/K chains: one [128,512] psum accumulated over all hc chunks
            for gi in range(2 * nl):
                is_q = gi % 2 == 0          # interleave Q,K per head
                t = gi // 2
                w_sb = wq_sb if is_q else wk_sb
                ps = psCH.tile([128, 512], FP, tag="chain", name="ps_qk")
                for c in range(hc):
                    nc.tensor.matmul(
                        ps[:],
                        w_sb[:, c, 128 * t:128 * t + 128],
                        x_sb[:, c, :],
                        start=(c == 0), stop=(c == hc - 1),
                    )
                if is_q:
                    nc.scalar.activation(
                        qT[t][:, cs], ps[:],
                        mybir.ActivationFunctionType.Identity,
                        bias=bq_sb[:, t:t + 1],
                    )
                else:
                    nc.vector.tensor_scalar_add(kT[t][:, cs], ps[:], bk_sb[:, t:t + 1])
            # V chains: natural [s,d] layout, one per 128-row s-tile
            for sti in range(4):
                st_idx = 4 * strip + sti
                ps = psCH.tile([128, dg], FP, tag="chain", name="ps_v")
                for c in range(hc):
                    nc.tensor.matmul(
                        ps[:],
                        x_sb[:, c, 128 * sti:128 * sti + 128],
                        wv_sb[:, c, :],
                        start=(c == 0), stop=(c == hc - 1),
                    )
                nc.vector.tensor_add(vv[st_idx][:], ps[:], bv_sb[:])

        def emit_attention_head(strip, l):
            """Causal attention for head l restricted to q-strip `strip`."""
            nk = 4 * strip + 4
            ps_av = psAV.tile([128, 512], FP, tag="ps_av", name="ps_av")
            dn_acc = dnp.tile([128, 512], FR, tag="dn_acc", name="dn_acc")
            ets = []
            # software-pipelined: scores(kt) ... AV(kt-1) so exp can run ahead
            for kt in range(nk + 1):
                if kt < nk:
                    c = kt - 4 * strip          # >=0 on diagonal tiles
                    qc = slice(128 * c, 512) if c >= 0 else slice(0, 512)
                    qg = slice(512 * strip + qc.start, 512 * strip + 512)
                    ps_s = psS.tile([128, 512], FP, tag="ps_s", name="ps_s")
                    nc.tensor.matmul(
                        ps_s[:, qc],
                        kT[l][:, 128 * kt:128 * kt + 128],
                        qT[l][:, qg],
                        start=True, stop=True,
                    )
                    et = etp.tile([128, 512], BF, tag="et", name="et")
                    nc.scalar.activation(
                        et[:, qc], ps_s[:, qc],
                        mybir.ActivationFunctionType.Exp,
                        scale=scale,
                    )
                    if c >= 0:
                        nc.vector.tensor_mul(
                            et[:, qc], et[:, qc], mask_sb[:, 384:896 - 128 * c])
                    # denominator accumulate on DVE
                    if kt == 0:
                        nc.vector.tensor_copy(dn_acc[:, qc], et[:, qc])
                    else:
                        nc.vector.tensor_add(dn_acc[:, qc], dn_acc[:, qc], et[:, qc])
                    ets.append((et, qc))
                if kt >= 1:
                    et, qc = ets[kt - 1]
                    nc.tensor.matmul(
                        ps_av[:, qc],
                        vv[kt - 1][:, 128 * l:128 * l + 128],
                        et[:, qc],
                        start=(kt - 1 == 0), stop=(kt - 1 == nk - 1),
                    )
            # denominator: ones-contract over partitions (PE), fast approx
            # reciprocal on the [1,512] row (DVE), ones-broadcast back to
            # [128,512] (PE), then normalize. ACT stages the PSUM->SBUF hops.
            ps_dn = psDR.tile([1, 512], FP, tag="dnrb", name="ps_dn")
            nc.tensor.matmul(ps_dn[:], ones_col, dn_acc[:], start=True, stop=True)
            dn_f32 = dnp.tile([1, 512], FP, tag="dn_f32", name="dn_f32")
            nc.scalar.copy(dn_f32[:], ps_dn[:])
            rec_f32 = dnp.tile([1, 512], FP, tag="rec_f32", name="rec_f32")
            nc.vector.reciprocal_approx_fast(rec_f32[:], dn_f32[:])
            rec_fr = dnp.tile([1, 512], FR, tag="rec_fr", name="rec_fr")
            nc.vector.tensor_copy(rec_fr[:], rec_f32[:])
            ps_rb = psDR.tile([128, 512], FP, tag="dnrb", name="ps_rb")
            nc.tensor.matmul(ps_rb[:], ones_row, rec_fr[:], start=True, stop=True)
            rb_sb = dnp.tile([128, 512], FP, tag="rb_sb", name="rb_sb", bufs=1)
            nc.scalar.copy(rb_sb[:], ps_rb[:])
            an = anp.tile([128, 512], F16, tag="an", name="an")
            nc.vector.tensor_mul(an[:], ps_av[:], rb_sb[:])
            # ship this head's strip slice to the AG input buffer
            nc.sync.dma_start(agin[strip][128 * l:128 * l + 128, :], an[:])

        def emit_ag(strip):
            if ag:
                nc.gpsimd.collective_compute(
                    "AllGather",
                    mybir.AluOpType.bypass,
                    replica_groups=REPLICA_GROUPS,
                    ins=[agin[strip].opt()],
                    outs=[agout[strip].opt()],
                )
            else:
                nc.gpsimd.dma_start(agout[strip][0:512, :], agin[strip][:])

        at_r = {}

        def emit_atr_loads(strip):
            """Fetch the 4 rank blocks of agout[strip] into SBUF.

            On the gpsimd queue, right after the AG trigger: these are the
            only consumers of the collective, so any completion-wait they
            carry blocks nothing else."""
            for r in range(GROUPS):
                t = atrp.tile([128, nl, 512], F16, tag=f"atr{r}", name=f"atr{r}")
                nc.gpsimd.dma_start(
                    t[:],
                    agout[strip][512 * r:512 * r + 512, :]
                    .rearrange("(l p) t -> p l t", p=128))
                at_r[(strip, r)] = t

        def emit_outproj_sti(strip, sti):
            """One 128-row s-tile of the output projection for `strip`."""
            rs = slice(512 * strip + 128 * sti, 512 * strip + 128 * sti + 128)
            ssl = slice(128 * sti, 128 * sti + 128)
            ps_o = psO.tile([128, dg], FP, tag="ps_o", name="ps_o")
            for c in range(hc):
                r, l = divmod(c, nl)
                lhsT = at_r[(strip, r)][:, l, ssl]
                nc.tensor.matmul(
                    ps_o[:], lhsT, wo_sb[:, c, :],
                    start=(c == 0), stop=(c == hc - 1),
                )
            ob = obp.tile([128, dg], FP, tag="ob", name="ob")
            nc.vector.tensor_add(ob[:], ps_o[:], bo_sb[:])
            nc.sync.dma_start(out_d[rs, :], ob[:])

        # ---- main fused pipeline ---------------------------------------
        # outproj for strip s runs during strip s+2 so even a slow AllGather
        # never head-of-line-blocks the PE queue.
        for strip in range(sq):
            if strip + 1 < sq:
                emit_x_load(strip + 1)
            emit_qkv(strip)
            for l in range(nl):
                emit_attention_head(strip, l)
                if strip >= 2:
                    emit_outproj_sti(strip - 2, l)
            emit_ag(strip)
            emit_atr_loads(strip)
        for strip in (sq - 2, sq - 1):
            for sti in range(4):
                emit_outproj_sti(strip, sti)


def make_inputs(x, W_qkv, b_qkv, W_out, b_out, s=S, h=H, nh=NH):
    """Host-side sharding: per-core input dicts (layout prep only)."""
    nl = nh // GROUPS
    dg = nl * D
    hc = h // 128
    sq = s // 512
    bf16 = ml_dtypes.bfloat16
    x = np.asarray(x, dtype=np.float32)
    W_qkv = np.asarray(W_qkv, dtype=np.float32)
    b_qkv = np.asarray(b_qkv, dtype=np.float32)
    W_out = np.asarray(W_out, dtype=np.float32)
    b_out = np.asarray(b_out, dtype=np.float32)

    # causal staircase master mask: mask[i, u] = 1 iff u >= i + 384
    uu = np.arange(896)[None, :]
    ii = np.arange(128)[:, None]
    mask = (uu >= ii + 384).astype(bf16)
    ones = np.ones((128, 128), dtype=np.float32)

    WoT = W_out.T  # [h (d-in), h (n-out)]
    in_maps = []
    for core in range(NCORES):
        b, g = divmod(core, GROUPS)
        xT = x[b].T                                   # [h, s]
        # x strips: [sq, 128, hc, 512]
        xs = np.ascontiguousarray(
            xT.reshape(hc, 128, sq, 512).transpose(2, 1, 0, 3).astype(bf16))

        def arr_w(wslice, dt):
            # [dg, h] -> transposed chunks [128, hc, dg]
            return np.ascontiguousarray(
                wslice.T.reshape(hc, 128, dg).transpose(1, 0, 2).astype(dt))

        wq = arr_w(W_qkv[dg * g:dg * (g + 1), :], bf16)
        wk = arr_w(W_qkv[h + dg * g:h + dg * (g + 1), :], bf16)
        wv = arr_w(W_qkv[2 * h + dg * g:2 * h + dg * (g + 1), :], bf16)
        bq = np.ascontiguousarray(
            b_qkv[dg * g:dg * (g + 1)].reshape(nl, 128).T)      # [128, nl]
        bk = np.ascontiguousarray(
            b_qkv[h + dg * g:h + dg * (g + 1)].reshape(nl, 128).T)
        bv = np.tile(b_qkv[2 * h + dg * g:2 * h + dg * (g + 1)][None, :], (128, 1))
        bo = np.tile(b_out[dg * g:dg * (g + 1)][None, :], (128, 1))
        # W_out^T rows permuted to the AG d-order: chunk c = 4r+l -> head 4r+l
        blocks = []
        for r in range(GROUPS):
            for l in range(nl):
                hh = nl * r + l
                blocks.append(WoT[D * hh:D * (hh + 1), dg * g:dg * (g + 1)])
        wo = np.ascontiguousarray(
            np.concatenate(blocks, axis=0)
            .reshape(hc, 128, dg).transpose(1, 0, 2).astype(np.float16))
        in_maps.append({
            "x": xs, "wq": wq, "wk": wk, "wv": wv, "wo": wo,
            "bq": bq, "bk": bk,
            "bv": np.ascontiguousarray(bv.astype(np.float32)),
            "bo": np.ascontiguousarray(bo.astype(np.float32)),
            "mask": mask, "ones": ones,
        })
    return in_maps


_NC_CACHE = {}


def _get_nc(key=(S, H, NH)):
    if key not in _NC_CACHE:
        _NC_CACHE[key] = build_nc(*key)
    return _NC_CACHE[key]


def kernel(x, W_qkv, b_qkv, W_out, b_out):
    global LAST_EXEC_NS, LAST_RESULTS
    nc = _get_nc()
    in_maps = make_inputs(x, W_qkv, b_qkv, W_out, b_out)
    res = run_bass_kernel_spmd(
        nc, in_maps, core_ids=list(range(NCORES)), trace=TRACE)
    LAST_EXEC_NS = res.exec_time_ns
    LAST_RESULTS = res
    out = np.empty((B, S, H), dtype=np.float32)
    for core in range(NCORES):
        b, g = divmod(core, GROUPS)
        out[b, :, DG * g:DG * (g + 1)] = res.results[core]["out"]
    return out
